# revision 38
# baseline (speedup 1.0000x reference)
"""Trainium2 Bass kernel for nn_BaseEncoder (ragged entity-pair encoder).

Contract: kernel(**inputs) takes the FULL unsharded inputs (numpy) and
returns the FULL output [B, Q, E, E, R] float32.

Sharding: B*Q = 8 independent (batch, query) pairs -> one per NeuronCore.
Small weights (W_head / W_tail / prototypes-for-that-b) are replicated.

Host-side prep per core (cheap, index/layout only):
  - gather the E*M mention rows of the per-query attention and sum over the
    M=2 mentions (the /2 and /NH scalings cancel in the later row-softmax-
    style normalization, so they are dropped),
  - transpose to At[l, (h,e)]; also send At2x with each e-column duplicated
    so the device multiplies two step-1 fp16 operands (DVE 2x mode),
  - S[e,f] = sum_{l,h} At[l,h,e]*At[l,h,f] and recs = 16/S (the 16 keeps
    recs in fp16 normal range; compensated by scaling W[H:] rows by 1/16),
  - entity means ent = mean_m seq[pos] (transposed to entT),
  - prototypes for this b, reshaped/transposed to [2H, R*P].

Device kernel per core (fp16 data, fp32 PSUM accumulation):
  prod[l,h,e,f] = At2x[l,h,e,.]*At[l,h,f]  (VectorE, fused packed-pair 2x)
  (chunk 1 computes only its (e>=16, f>=16) quadrant; the mirrored half is
   filled at the normalized-ctx level from chunk 0 -- ctx/S is symmetric)
  mul[l,ef] = sum_h prod                   (VectorE tree adds, 2x)
  ctxT[h',ef] = sum_l seq[l,h'] mul[l,ef]          (TensorE)
  cn = ctxT * recs                                  (ScalarE copy + VectorE)
  epT[e,h''] = sum_h' entT[h',e] W[h',h'']          (TensorE)
  pre[h'',ef] = sum_kt W[H+kt,h''] cn[kt,ef] + mask-fold of epT   (TensorE)
  cand = tanh(pre)                                  (ScalarE, from PSUM)
  scores[rp,ef] = sum_d candT[d,ef] protoT[d,rp]    (TensorE)
  out[ef,r] = max_p scores                          (transpose + VectorE)
"""

import numpy as np

B, Q, L, H, E, M, R, P, NH = 2, 4, 1024, 768, 32, 2, 5, 10, 12
NCORES = 8
LT = L // 128          # 8 l-tiles
HT = H // 128          # 6 tiles of 128 along a hidden dim
EF = E * E             # 1024 entity pairs
RP = R * P             # 50 prototype rows
EC = E // 2            # 16 e-rows per chunk
HC = EF // 2           # 512-wide ef chunk (= one PSUM bank of fp32)

_CACHE = {}


def _build_program():
    import concourse.mybir as mybir
    import concourse.tile as tile
    from concourse import bacc

    f16 = mybir.dt.float16
    f32 = mybir.dt.float32
    nc = bacc.Bacc("TRN2", target_bir_lowering=False, debug=False,
                   num_devices=NCORES)

    at_d = nc.dram_tensor("at", [L, NH * E], f16, kind="ExternalInput").ap()
    # at2: e-duplicated pairs with the chunk outermost so per-chunk slices
    # stay contiguous: at2[l, (c, h, el, 2)] = at[l, (h, 16c+el)]
    at2_d = nc.dram_tensor("at2", [L, NH * E * 2], f16,
                           kind="ExternalInput").ap()
    seq_d = nc.dram_tensor("seq", [L, H], f16, kind="ExternalInput").ap()
    entT_d = nc.dram_tensor("entT", [H, E], f16, kind="ExternalInput").ap()
    wh_d = nc.dram_tensor("wh", [2 * H, H], f16, kind="ExternalInput").ap()
    wt_d = nc.dram_tensor("wt", [2 * H, H], f16, kind="ExternalInput").ap()
    ptT_d = nc.dram_tensor("ptT", [2 * H, RP], f16, kind="ExternalInput").ap()
    recs_d = nc.dram_tensor("recs", [1, EF], f16, kind="ExternalInput").ap()
    out_d = nc.dram_tensor("out", [EF, R], f32, kind="ExternalOutput").ap()

    with tile.TileContext(nc) as tc:
        _emit(tc, mybir, at_d, at2_d, seq_d, entT_d, wh_d, wt_d, ptT_d,
              recs_d, out_d)

    nc.compile()
    return nc


def _emit(tc, mybir, at_d, at2_d, seq_d, entT_d, wh_d, wt_d, ptT_d, recs_d,
          out_d):
    nc = tc.nc
    f16 = mybir.dt.float16
    f32 = mybir.dt.float32

    Alu = mybir.AluOpType
    Act = mybir.ActivationFunctionType
    Ax = mybir.AxisListType
    from concourse.masks import make_identity

    import contextlib
    ctx = contextlib.ExitStack()
    with ctx:
        const = ctx.enter_context(tc.tile_pool(name="const", bufs=1))
        big = ctx.enter_context(tc.tile_pool(name="big", bufs=1))
        mulp = ctx.enter_context(tc.tile_pool(name="mulp", bufs=16))
        candp = ctx.enter_context(tc.tile_pool(name="candp", bufs=14))
        ctxp = ctx.enter_context(tc.tile_pool(name="ctxp", bufs=2))
        tmp = ctx.enter_context(tc.tile_pool(name="tmp", bufs=2))
        # PSUM: 8 banks statically split into tags
        #   "ctx": 6 x 1 bank   (per-chunk ctx accumulators; later proj-B)
        #   "sg":  1 x 1 bank   (even proj-A groups, transposes)
        #   "tail": 1 x 1 bank  (epT, odd proj-A groups, scores)
        psum = ctx.enter_context(tc.tile_pool(name="psum", bufs=1,
                                              space="PSUM"))

        # ---------------- input loads (per-lt interleaved) ----------------
        at_sb = big.tile([128, LT, NH * E], f16, tag="at_sb")
        at2_sb = big.tile([128, LT, NH * E * 2], f16, tag="at2_sb")
        seq_sb = big.tile([128, LT, H], f16, tag="seq_sb")
        at_r = at_d.rearrange("(t p) n -> p t n", p=128)
        at2_r = at2_d.rearrange("(t p) n -> p t n", p=128)
        seq_r = seq_d.rearrange("(t p) n -> p t n", p=128)
        for lt in range(3):
            nc.sync.dma_start(out=at_sb[:, lt, :], in_=at_r[:, lt, :])
            nc.sync.dma_start(out=at2_sb[:, lt, :], in_=at2_r[:, lt, :])
        for k in range(LT):
            nc.sync.dma_start(out=seq_sb[:, k, :], in_=seq_r[:, k, :])
            if k + 3 < LT:
                lt = k + 3
                nc.sync.dma_start(out=at_sb[:, lt, :], in_=at_r[:, lt, :])
                nc.sync.dma_start(out=at2_sb[:, lt, :],
                                  in_=at2_r[:, lt, :])
        # bulk tensors go on the second HWDGE queue (Activation engine) so
        # they don't delay the latency-critical per-lt at/at2 stream above
        entT_sb = const.tile([128, HT, E], f16, tag="entT_sb")
        nc.sync.dma_start(out=entT_sb, in_=entT_d.rearrange(
            "(t p) n -> p t n", p=128))
        # recs broadcast to all 128 partitions straight from the DMA
        recS_sb = big.tile([128, EF], f16, tag="recS_sb")
        nc.sync.dma_start(out=recS_sb, in_=recs_d.partition_broadcast(128))
        wh_sb = big.tile([128, 2 * HT, H], f16, tag="wh_sb")
        nc.sync.dma_start(out=wh_sb, in_=wh_d.rearrange(
            "(t p) n -> p t n", p=128))
        wt_sb = big.tile([128, 2 * HT, H], f16, tag="wt_sb")
        nc.sync.dma_start(out=wt_sb, in_=wt_d.rearrange(
            "(t p) n -> p t n", p=128))
        ptT_sb = const.tile([128, 2 * HT, RP], f16, tag="ptT_sb")
        nc.sync.dma_start(out=ptT_sb, in_=ptT_d.rearrange(
            "(t p) n -> p t n", p=128))

        # ---------------- constants: identities and bias masks ----------
        ident32 = const.tile([E, E], f16, tag="ident32")
        make_identity(nc, ident32)
        identRP = const.tile([RP, RP], f32, tag="identRP")
        make_identity(nc, identRP)
        # mask_h[c][e', (el,f)] = 1 iff e' == 16c+el ; mask_t[f',(el,f)] =
        # 1 iff f'==f. Rows >= 32 are zero so the epT stationary rows
        # beyond 32 contribute nothing.
        mask_h = []
        for c in range(2):
            mk = const.tile([128, HC], f16, tag=f"mask_h{c}")
            nc.gpsimd.memset(mk, 0.0)
            nc.scalar.copy(
                mk[0:E, :].rearrange("p (e f) -> p e f", e=EC),
                ident32[:, c * EC:(c + 1) * EC, None].broadcast_to(
                    [E, EC, E]))
            mask_h.append(mk)
        mask_t = const.tile([128, HC], f16, tag="mask_t")
        nc.gpsimd.memset(mask_t, 0.0)
        nc.scalar.copy(
            mask_t[0:E, :].rearrange("p (e f) -> p e f", e=EC),
            ident32[:, None, :].broadcast_to([E, EC, E]))

        # ---------------- entity projections epT[e, h''] ------------------
        # epT_w = entT^T(W_w[:H]) : stationary entT [h'-part, e], moving W.
        epT_sb = const.tile([128, 2, H], f16, tag="epT_sb")
        nc.gpsimd.memset(epT_sb, 0.0)

        def emit_epT():
            HH = H // 2
            for w, wsb in ((0, wh_sb), (1, wt_sb)):
                for half in range(2):
                    ps = psum.tile([E, HH], f32, tag="tail", bufs=1,
                                   name=f"epT{w}_{half}")
                    for kt in range(HT):
                        nc.tensor.matmul(
                            ps, entT_sb[:, kt, :],
                            wsb[:, kt, half * HH:(half + 1) * HH],
                            start=(kt == 0), stop=(kt == HT - 1))
                    nc.scalar.copy(
                        epT_sb[0:E, w, half * HH:(half + 1) * HH], ps)

        # ---------------- chunked main pipeline ----------------
        # Chunk c covers pairs ef in [c*512, (c+1)*512) i.e. e in
        # [16c, 16c+16).  prod[l,h,e,f] computed as packed fp16 pairs so the
        # DVE runs in 2x mode; h-sum tree: L1 on DVE, L2 on GpSimd
        # (software-pipelined one lt behind), L3 back on DVE.

        def emit_prod(c, lt):
            at3 = at_sb[:, lt, :].rearrange("p (h e) -> p h e", h=NH)
            at4 = at2_sb[:, lt, :].rearrange("p (c h e two) -> p c h e two",
                                             c=2, h=NH, two=2)
            fs = 0 if c == 0 else EC
            FW = E - fs
            pr = tmp.tile([128, NH, EC, FW], f16, tag=f"prod{c}",
                          name=f"prod{c}_{lt}")
            in1 = at4[:, c][:, :, :, None, :].broadcast_to(
                [128, NH, EC, FW // 2, 2])
            in2 = at3[:, :, fs:].rearrange(
                "p h (fh fl) -> p h fh fl", fl=2)[:, :, None, :, :]
            in2 = in2.broadcast_to([128, NH, EC, FW // 2, 2])
            nc.vector.tensor_mul(
                pr.rearrange("p h e (fh fl) -> p h e fh fl", fl=2), in1, in2)
            # h-sum tree: L1 12->6, L2 6->3 (both DVE; GpSimd is ~2x slower
            # and contends for the shared SBUF port)
            nc.vector.tensor_add(pr[:, 0:6], pr[:, 0:6], pr[:, 6:12])
            nc.vector.tensor_add(pr[:, 0:3], pr[:, 0:3], pr[:, 3:6])
            return pr

        def emit_mul_fin(c, lt, pr, mt):
            """L3 of the h-sum tree (DVE) -> mul tile (dense)."""
            m3 = mt.rearrange("p (e f) -> p e f", e=EC)
            nc.vector.tensor_add(m3, pr[:, 0], pr[:, 1])
            nc.vector.tensor_add(m3, m3, pr[:, 2])

        def emit_ctx_chunk(c, lt, mt, ctx_ps):
            for ht in range(HT):
                nc.tensor.matmul(
                    ctx_ps[ht], seq_sb[:, lt, ht * 128:(ht + 1) * 128],
                    mt, start=(lt == 0), stop=(lt == LT - 1))

        def emit_norm_chunk(c, ctx_ps):
            cn = ctxp.tile([128, HT, HC], f16, tag="ctxn", name=f"ctxn{c}")
            cc = tmp.tile([128, HT, HC], f16, tag="ctxc", name=f"ctxc{c}")
            for ht in range(HT):
                nc.scalar.copy(cc[:, ht, :], ctx_ps[ht])
                nc.vector.tensor_mul(cn[:, ht, :], cc[:, ht, :],
                                     recS_sb[:, c * HC:(c + 1) * HC])
            return cn

        def emit_norm_chunk1(ctx_ps, cnA):
            """Chunk-1 norm, quadrant cols only: the mirrored cols (f<16)
            were filled right after norm-A (ctx/S is symmetric)."""
            cn = cn1
            cc = tmp.tile([128, HT, EC * EC], f16, tag="ctxc1", name="ctxc1")
            rq = recS_sb[:, HC:].rearrange("p (e f) -> p e f", e=EC)[:, :, EC:]
            for ht in range(HT):
                nc.scalar.copy(cc[:, ht, :], ctx_ps[ht])
                cnv = cn[:, ht, :].rearrange("p (e f) -> p e f", e=EC)
                nc.vector.tensor_mul(
                    cnv[:, :, EC:],
                    cc[:, ht, :].rearrange("p (e f) -> p e f", e=EC), rq)
            return cn

        def emit_proj_group(c, g, cn, cand_t, ps_tag, sc=None):
            w, ht2 = divmod(g, HT)
            wsb = wh_sb if w == 0 else wt_sb
            nb = HT if ps_tag == "ctx" else 1
            ps = psum.tile([128, HC], f32, tag=ps_tag, bufs=nb,
                           name=f"proj{c}_{g}")
            for kt in range(HT):
                nc.tensor.matmul(ps, wsb[:, HT + kt,
                                         ht2 * 128:(ht2 + 1) * 128],
                                 cn[:, kt, :],
                                 start=(kt == 0), stop=False)
            # bias fold: += epT_w[sel(ef), h''] via the 0/1 mask moving
            mk = mask_h[c] if w == 0 else mask_t
            nc.tensor.matmul(ps, epT_sb[:, w, ht2 * 128:(ht2 + 1) * 128],
                             mk, start=False, stop=True)
            cd = candp.tile([128, HC], f16, tag="cand", name=f"cand{c}_{g}")
            cand_t[g] = cd
            nc.scalar.activation(cd, ps, Act.Tanh)
            if sc is not None:
                # interleave this chunk's scores accumulation step
                nc.tensor.matmul(sc, ptT_sb[:, g, :], cd,
                                 start=(g == 0), stop=(g == 2 * HT - 1))

        def emit_scores_steps(c, cand_t, sc):
            order = [w * HT + kt for w in range(2) for kt in range(HT)]
            for i, g in enumerate(order):
                nc.tensor.matmul(sc, ptT_sb[:, g, :], cand_t[g],
                                 start=(i == 0), stop=(i == 2 * HT - 1))

        def emit_scores_tail(c, sc):
            scT = const.tile([RP, HC], f32, tag=f"scT{c}", name=f"scT{c}")
            nc.scalar.copy(scT, sc)
            ob = const.tile([128, LT // 2, R], f32, tag=f"ob{c}",
                            name=f"ob{c}")
            for et in range(LT // 2):
                tp = psum.tile([128, RP], f32, tag="sg", bufs=1, name="tp")
                nc.tensor.transpose(tp, scT[:, et * 128:(et + 1) * 128],
                                    identRP)
                nc.vector.tensor_reduce(
                    out=ob[:, et, :],
                    in_=tp.rearrange("p (r q) -> p r q", r=R),
                    axis=Ax.X, op=Alu.max)
            nc.sync.dma_start(
                out=out_d.rearrange("(t p) r -> p t r", p=128)[
                    :, c * (LT // 2):(c + 1) * (LT // 2), :],
                in_=ob)

        # ---- phase A: mul+ctx for chunk 0 (L3+ctx pipelined 1 lt behind) --
        ctxA_ps = [psum.tile([128, HC], f32, tag="ctx", bufs=HT,
                             name=f"ctxA{ht}") for ht in range(HT)]
        mulA_t = [mulp.tile([128, HC], f16, tag="mulA", bufs=8,
                            name=f"mulA_{lt}") for lt in range(LT)]
        pend = None
        for lt in range(LT):
            pr = emit_prod(0, lt)
            if pend is not None:
                plt, ppr = pend
                emit_mul_fin(0, plt, ppr, mulA_t[plt])
                emit_ctx_chunk(0, plt, mulA_t[plt], ctxA_ps)
            pend = (lt, pr)
            if lt == 3:
                emit_epT()
        plt, ppr = pend
        emit_mul_fin(0, plt, ppr, mulA_t[plt])
        emit_ctx_chunk(0, plt, mulA_t[plt], ctxA_ps)
        cnA = emit_norm_chunk(0, ctxA_ps)
        # chunk-1 mirrored cols (f<16) depend only on cnA -- fill them now,
        # off the tail-critical path: cn1[el, f1] = cnA[f1, 16+el]
        cn1 = ctxp.tile([128, HT, HC], f16, tag="ctxn", name="ctxn1")
        for ht in range(HT):
            wv = cnA[:, ht, :].rearrange("p (e f) -> p e f", e=EC)[:, :, EC:]
            nc.scalar.copy(
                cn1[:, ht, :].rearrange("p (e f) -> p e f", e=EC)[:, :, :EC],
                wv.rearrange("p a b -> p b a"))

        # ---- phase B: mul+ctx for chunk 1 (quadrant only), interleaved
        # with the chunk-0 tail ----
        candA = [None] * (2 * HT)
        ctxB_ps = [psum.tile([128, EC * EC], f32, tag="ctx", bufs=HT,
                             name=f"ctxB{ht}") for ht in range(HT)]
        mulB_t = [mulp.tile([128, EC * EC], f16, tag="mulB", bufs=8,
                            name=f"mulB_{lt}") for lt in range(LT)]
        projA_sched = {0: [0, 1], 1: [2, 3], 2: [4, 5], 3: [6, 7],
                       4: [8, 9], 5: [10, 11]}
        pend = None
        for lt in range(LT):
            pr = emit_prod(1, lt)
            if pend is not None:
                plt, ppr = pend
                emit_mul_fin(1, plt, ppr, mulB_t[plt])
                emit_ctx_chunk(1, plt, mulB_t[plt], ctxB_ps)
            pend = (lt, pr)
            for g in projA_sched.get(lt, []):
                emit_proj_group(0, g, cnA, candA, "sg" if g % 2 == 0
                                else "tail")
        plt, ppr = pend
        emit_mul_fin(1, plt, ppr, mulB_t[plt])
        emit_ctx_chunk(1, plt, mulB_t[plt], ctxB_ps)
        scA = psum.tile([RP, HC], f32, tag="tail", bufs=1, name="scA")
        emit_scores_steps(0, candA, scA)
        cnB = emit_norm_chunk1(ctxB_ps, cnA)
        emit_scores_tail(0, scA)

        # ---- chunk-1 tail (PE slots from the freed ctx accumulators) ----
        candB = [None] * (2 * HT)
        scB = psum.tile([RP, HC], f32, tag="tail", bufs=1, name="scB")
        for g in range(2 * HT):
            emit_proj_group(1, g, cnB, candB, "ctx", sc=scB)
        emit_scores_tail(1, scB)


def _host_prep(sequence_output, attention, W_head, W_tail, prototypes,
               mention_pos):
    """Build the per-core input maps (numpy only)."""
    seq = np.asarray(sequence_output, dtype=np.float32)
    att = np.asarray(attention, dtype=np.float32)
    wh = np.asarray(W_head, dtype=np.float32).copy()
    wt = np.asarray(W_tail, dtype=np.float32).copy()
    # the device normalizer is recs = 16/S (fp16-range safe); compensate by
    # scaling the ctx-rows of the projection weights by 1/16.
    wh[H:] *= np.float32(1.0 / 16.0)
    wt[H:] *= np.float32(1.0 / 16.0)
    wh16 = np.ascontiguousarray(wh, dtype=np.float16)
    wt16 = np.ascontiguousarray(wt, dtype=np.float16)
    pro = np.asarray(prototypes, dtype=np.float32)
    pos = np.asarray(mention_pos)

    in_maps = []
    for c in range(NCORES):
        b, q = divmod(c, Q)
        p_bq = pos[b, q]                       # [E, M]
        # attention gather + mention-sum: [NH, E, L] (scale dropped)
        g = att[b, q][:, p_bq, :]              # [NH, E, M, L]
        asum = g[:, :, 0, :] + g[:, :, 1, :]   # [NH, E, L]
        at = np.ascontiguousarray(
            asum.reshape(NH * E, L).T, dtype=np.float16)  # [L, NH*E]
        # at2[l, (c, h, el, 2)] = at[l, (h, 16c+el)], chunk-outermost
        at2 = np.ascontiguousarray(
            np.repeat(at.reshape(L, NH, 2, EC).transpose(0, 2, 1, 3), 2,
                      axis=3).reshape(L, NH * E * 2))
        # normalizer S[e,f] = sum_{h,l} At[l,h,e] At[l,h,f]
        Bm = np.ascontiguousarray(
            asum.transpose(1, 0, 2).reshape(E, NH * L))
        S = Bm @ Bm.T                           # [E, E]
        recs = np.ascontiguousarray(
            (np.float32(16.0) / S).reshape(1, EF), dtype=np.float16)
        # entity means: [E, H] -> entT [H, E]
        ment = seq[b, q][p_bq]                 # [E, M, H]
        ent = (ment[:, 0, :] + ment[:, 1, :]) * np.float32(0.5)
        entT = np.ascontiguousarray(ent.T, dtype=np.float16)
        ptT = np.ascontiguousarray(
            pro[b].reshape(RP, 2 * H).T, dtype=np.float16)  # [2H, RP]
        in_maps.append({
            "at": at,
            "at2": at2,
            "seq": np.ascontiguousarray(seq[b, q], dtype=np.float16),
            "entT": entT,
            "wh": wh16,
            "wt": wt16,
            "ptT": ptT,
            "recs": recs,
        })
    return in_maps


def kernel(sequence_output, attention, W_head, W_tail, prototypes,
           mention_pos):
    from concourse.bass_utils import run_bass_kernel_spmd

    if "nc" not in _CACHE:
        _CACHE["nc"] = _build_program()
    nc = _CACHE["nc"]

    in_maps = _host_prep(sequence_output, attention, W_head, W_tail,
                         prototypes, mention_pos)
    res = run_bass_kernel_spmd(nc, in_maps, core_ids=list(range(NCORES)))

    out = np.empty((B, Q, E, E, R), dtype=np.float32)
    for c in range(NCORES):
        b, q = divmod(c, Q)
        out[b, q] = res.results[c]["out"].reshape(E, E, R)
    return out
